# revision 16
# baseline (speedup 1.0000x reference)
"""Trainium2 Bass kernel for nn_Attn (Luong 'general'-score attention softmax).

reference:
    energy[b,l,:] = targets[b,l,:] @ W.T + bias          # [B, L, H]
    s[b,l]        = energy[b,l,:] . h[b,:]               # [B, L]
    out           = softmax(s, axis=1)[:, None, :]       # [B, 1, L]

Algebraic refactor (exact up to fp rounding):
    s[b,l] = targets[b,l,:] . v[b,:] + const_b, with v[b,:] = h[b,:] @ W;
    const_b cancels in softmax.  v is computed on the HOST (0.01% of the
    flops).

fp8 screening + exact rescore:
    The kernel streams targets as fp8 e4m3 (halving HBM traffic vs fp16,
    which is the binding resource: ~358 GB/s per NeuronCore) and computes
    screening scores s8[b,l] on the PE (DoubleRow fp8 matmuls, fp32
    accumulation).  Scores have sigma ~32 across a row while the fp8
    quantization error is sigma ~1.2 (max ~6), so softmax is decided by
    the few rows within ~MARGIN of the row max.  The host rescores only
    those candidate rows exactly (float64) and computes the softmax; all
    other probabilities are < e^-MARGIN and their fp8 error is
    invisible at fp32 output precision.

Device program (per core, 4 batches):
    16 x 1 MiB fp8 chunk DMAs issued up-front, alternating across the two
    HWDGE rings (sync + scalar) so descriptor/completion overheads on one
    ring hide under the other ring's streaming; all 16 chunks are resident
    in SBUF (no recycling, so DMA never stalls on consumers).  PE consumes
    each chunk with v-stationary DoubleRow matmuls: lhsT = v8 [128,2,1]
    (an h-chunk pair of v), rhs = t8 [128,2,512] -> PSUM [1,512] per
    512-l block, 4 accumulating matmuls per block.  DVE drains PSUM to a
    [4, 4096] score tile; per-batch score rows stream back via SWDGE
    (gpsimd) so the HW rings stay dedicated to the input stream.
"""

import json

import ml_dtypes
import numpy as np

import concourse.bass as bass
import concourse.tile as tile
from concourse import bass2jax, bass_utils, mybir
from concourse.bass_utils import run_bass_kernel_spmd

F32 = mybir.dt.float32
F8 = mybir.dt.float8e4
E4 = ml_dtypes.float8_e4m3

B, L, H = 32, 4096, 1024
NCORES = 8
BPC = B // NCORES          # batches per core (4)
NCELL = 8                  # 512-l cells per batch (one PSUM block each)
NQ = 4                     # h-chunk pairs (DoubleRow: 2x128 contraction)
LB = 512                   # l per cell == PSUM bank capacity in fp32
CELLF = NQ * 2 * LB        # free elems per cell per partition (4096)
MARGIN = 24.0              # fp8 score error is sigma~1.2, max~6

# Transfer plan: (batch, cell_lo, cell_hi, queue).  2 MiB transfers for
# the bulk (fewer inter-transfer ring gaps), tapering to 512 KiB over
# the last 2 MiB so the final matmul+drain chain starts as early as
# possible.  The scalar (ACT) HWDGE ring observably starts ~3 us before
# the sync (SP) ring, so scalar leads each pair and carries the final
# transfer.  Emission order == consumption order.
TRANSFERS = [
    (0, 0, 4, "scalar"), (0, 4, 8, "sync"),
    (1, 0, 4, "scalar"), (1, 4, 8, "sync"),
    (2, 0, 4, "scalar"), (2, 4, 8, "sync"),
    (3, 0, 2, "scalar"), (3, 2, 4, "sync"),
    (3, 4, 5, "scalar"), (3, 5, 6, "sync"),
    (3, 6, 7, "sync"), (3, 7, 8, "scalar"),
]


def _split_multiwaits(bir_json):
    """The walrus build here lowers at most ONE sem-wait per instruction;
    hoist extra waits into standalone EventSemaphore instructions inserted
    just before the owner (same engine => same in-order stream)."""
    bir = json.loads(bir_json)
    for fn in bir["functions"]:
        for blk in fn["blocks"]:
            new_insts = []
            for ins in blk.get("instructions", []):
                si = ins.get("sync_info")
                ow = (si or {}).get("on_wait") or []
                if len(ow) > 1:
                    for k, w in enumerate(ow[:-1]):
                        new_insts.append(
                            {
                                "debug": ins.get("debug", 0),
                                "engine": ins["engine"],
                                "ins": [],
                                "name": f"{ins['name']}_hw{k}",
                                "opcode": "EventSemaphore",
                                "outs": [],
                                "sync_info": {"on_update": [], "on_wait": [w]},
                            }
                        )
                    si["on_wait"] = [ow[-1]]
                new_insts.append(ins)
            blk["instructions"] = new_insts
    return json.dumps(bir).encode()


_ORIG_COMPILE_BIR = bass_utils.compile_bir_kernel


def _compile_bir_split(bir_json, tmpdir, neff_name="file.neff"):
    return _ORIG_COMPILE_BIR(_split_multiwaits(bir_json), tmpdir, neff_name=neff_name)


def _patch_compile():
    bass_utils.compile_bir_kernel = _compile_bir_split
    bass2jax.compile_bir_kernel = _compile_bir_split


def _patch_tile_drain():
    """walrus in this env only lowers 1 sem-wait per TPB_CTRL Drain; split
    the TileContext exit-drain waits into individual wait_ge instructions."""
    if getattr(tile.TileContext, "_drain_patched", False):
        return

    def _drain_and_barrier(self, tick_clock, wait_clock):
        nc = self.nc
        drain_inst = nc.sync.drain()
        wait_clock.add_sem_waits(
            drain_inst.ins, tile.ScopedClock({None: tick_clock.global_clock})
        )
        si = drain_inst.ins.sync_info
        waits = list(si.on_wait or [])
        if len(waits) > 1:
            si.on_wait = []
            handles = {}
            for h in self.sems.allocated().values():
                handles[getattr(h, "name", None) or str(h)] = h
            for ww in waits:
                nc.sync.wait_ge(handles[ww.ant_name], ww.wait_value)
        nc.all_engine_barrier()
        popped = nc._tile_sem_poison_stack.pop()
        assert popped is self._sem_poison
        nc.clear_and_free_semaphores(list(self.sems.allocated().values()))
        nc.all_engine_barrier()

    tile.TileContext._drain_and_barrier = _drain_and_barrier
    tile.TileContext._drain_patched = True


def build_kernel(tc, t8d, v8d, outd):
    nc = tc.nc

    import contextlib

    ctx = contextlib.ExitStack()
    consts = ctx.enter_context(tc.tile_pool(name="consts", bufs=1))
    chp = ctx.enter_context(tc.tile_pool(name="chunks", bufs=BPC))
    sp = ctx.enter_context(tc.tile_pool(name="scores", bufs=1))
    psp = ctx.enter_context(tc.tile_pool(name="ps", bufs=4, space="PSUM"))

    # v8[p, two, b*NQ+q] = fp8(v[b, (q*2+two)*128 + p]).  The DoubleRow
    # LDWEIGHTS ISA requires the k-pair dim's step to be a multiple of 16
    # (bytes), hence pair-partner columns 16 apart rather than adjacent.
    v8 = consts.tile([128, 2, BPC * NQ], F8)
    nc.gpsimd.dma_start(out=v8, in_=v8d.rearrange("p (t i) -> p t i", t=2))
    # One score row per batch, each on partition 0 (engine APs must start
    # at a 32-aligned partition, so a [BPC, L] tile with per-batch rows
    # fails BIR verification).
    S = [sp.tile([1, L], F32, name=f"S{b}") for b in range(BPC)]

    # One resident tile per batch (4 MiB each, 16 MiB total); the DMA
    # transfers write disjoint cell ranges and the Tile framework
    # range-tracks, so matmuls wait only on the slice they read.
    tg = [
        chp.tile([128, NCELL, NQ, 2, LB], F8, tag="tg", name=f"t{b}")
        for b in range(BPC)
    ]
    for b, lo, hi, qname in TRANSFERS:
        eng = nc.sync if qname == "sync" else nc.scalar
        eng.dma_start(
            out=tg[b][:, lo:hi],
            in_=t8d[b][:, lo * CELLF : hi * CELLF].rearrange(
                "p (c q t l) -> p c q t l", q=NQ, t=2, l=LB
            ),
        )

    for b in range(BPC):
        for cell in range(NCELL):
            ps = psp.tile([1, LB], F32, tag="ps", name=f"ps{b}_{cell}")
            for q in range(NQ):
                idx = b * NQ + q
                nc.tensor.matmul(
                    ps,
                    lhsT=v8[:, :, idx : idx + 1],
                    rhs=tg[b][:, cell, q],
                    start=(q == 0),
                    stop=(q == NQ - 1),
                    perf_mode=mybir.MatmulPerfMode.DoubleRow,
                )
            col = cell * LB
            # Alternate PSUM drains between DVE and ACT so back-to-back
            # cell drains at the tail don't serialize on one engine.
            if cell % 2 == 0:
                nc.vector.tensor_copy(S[b][:, col : col + LB], ps)
            else:
                nc.scalar.activation(
                    out=S[b][:, col : col + LB],
                    in_=ps,
                    func=mybir.ActivationFunctionType.Copy,
                )
        # Batches 0-2 stream their score rows out on the idle SWDGE path
        # (done long before the tail); batch 3 goes on the sync HWDGE
        # ring (empty right after its last chunk) in two pieces so the
        # end-gating store is only the last cell's 2 KiB.
        if b < BPC - 1:
            nc.gpsimd.dma_start(out=outd[b], in_=S[b][:, :])
    cut = (NCELL - 1) * LB
    nc.sync.dma_start(out=outd[BPC - 1][0:cut], in_=S[BPC - 1][:, 0:cut])
    nc.sync.dma_start(out=outd[BPC - 1][cut:L], in_=S[BPC - 1][:, cut:L])
    ctx.close()


def build_bass():
    _patch_tile_drain()
    _patch_compile()
    nc = bass.Bass("TRN2", target_bir_lowering=False, debug=False, num_devices=NCORES)
    t8d = nc.dram_tensor(
        "t8", [BPC, 128, NCELL * CELLF], F8, kind="ExternalInput"
    ).ap()
    v8d = nc.dram_tensor("v8", [128, BPC * NQ * 2], F8, kind="ExternalInput").ap()
    outd = nc.dram_tensor("out", [BPC, L], F32, kind="ExternalOutput").ap()
    with tile.TileContext(nc) as tc:
        build_kernel(tc, t8d, v8d, outd)
    return nc


def make_in_maps(hidden, targets, W):
    h64 = hidden[0].astype(np.float64)                    # [B, H]
    v8 = (h64 @ W.astype(np.float64)).astype(np.float32).astype(E4)  # [B, H]

    in_maps = []
    for c in range(NCORES):
        tl = targets[c * BPC : (c + 1) * BPC]             # [4, 4096, 1024] f32
        t8 = tl.astype(E4)
        # l = cell*512 + l' ; h = (q*2+t)*128 + p
        t8r = t8.reshape(BPC, NCELL, LB, NQ, 2, 128)      # [b,cell,l',q,t,p]
        t8r = np.ascontiguousarray(t8r.transpose(0, 5, 1, 3, 4, 2))
        t8c = t8r.reshape(BPC, 128, NCELL * CELLF)
        vloc = v8[c * BPC : (c + 1) * BPC]                # [4, 1024]
        # v8c[p, t*16 + b*NQ + q] = vloc[b, (q*2+t)*128 + p]
        v8c = np.ascontiguousarray(
            vloc.reshape(BPC, NQ, 2, 128).transpose(3, 2, 0, 1)
        ).reshape(128, 2 * BPC * NQ)
        in_maps.append({"t8": t8c, "v8": v8c})
    return in_maps


_CACHED_NC = None


def kernel(hidden, targets, W, b, _trace=False):
    global _CACHED_NC
    if _CACHED_NC is None:
        _CACHED_NC = build_bass()
    nc = _CACHED_NC
    in_maps = make_in_maps(hidden, targets, W)
    res = run_bass_kernel_spmd(nc, in_maps, list(range(NCORES)), trace=_trace)
    s8 = np.concatenate([res.results[c]["out"] for c in range(NCORES)], axis=0)
    kernel.last_results = res

    # Host: exact rescore of candidate rows (those within MARGIN of the
    # row max -- typically ~10 of 4096) + float64 softmax.
    h64 = hidden[0].astype(np.float64)
    v64 = h64 @ W.astype(np.float64)                      # [B, H]
    out = np.empty((B, 1, L), np.float32)
    sc = s8.astype(np.float64)
    for bb in range(B):
        row = sc[bb]
        cand = np.flatnonzero(row >= row.max() - MARGIN)
        row[cand] = targets[bb, cand].astype(np.float64) @ v64[bb]
        e = np.exp(row - row.max())
        out[bb, 0] = (e / e.sum()).astype(np.float32)
    return out


# revision 19
# speedup vs baseline: 1.1437x; 1.1437x over previous
"""Trainium2 Bass kernel for nn_Attn (Luong 'general'-score attention softmax).

reference:
    energy[b,l,:] = targets[b,l,:] @ W.T + bias          # [B, L, H]
    s[b,l]        = energy[b,l,:] . h[b,:]               # [B, L]
    out           = softmax(s, axis=1)[:, None, :]       # [B, 1, L]

Algebraic refactor (exact up to fp rounding):
    s[b,l] = targets[b,l,:] . v[b,:] + const_b, with v[b,:] = h[b,:] @ W;
    const_b cancels in softmax.  v is computed on the HOST (0.01% of the
    flops).

fp8 screening + exact rescore:
    The kernel streams targets as fp8 e4m3 (halving HBM traffic vs fp16,
    which is the binding resource: ~358 GB/s per NeuronCore) and computes
    screening scores s8[b,l] on the PE (DoubleRow fp8 matmuls, fp32
    accumulation).  Scores have sigma ~32 across a row while the fp8
    quantization error is sigma ~1.2 (max ~6), so softmax is decided by
    the few rows within ~MARGIN of the row max.  The host rescores only
    those candidate rows exactly (float64) and computes the softmax; all
    other probabilities are < e^-MARGIN and their fp8 error is
    invisible at fp32 output precision.

Device program (per core, 4 batches):
    16 x 1 MiB fp8 chunk DMAs issued up-front, alternating across the two
    HWDGE rings (sync + scalar) so descriptor/completion overheads on one
    ring hide under the other ring's streaming; all 16 chunks are resident
    in SBUF (no recycling, so DMA never stalls on consumers).  PE consumes
    each chunk with v-stationary DoubleRow matmuls: lhsT = v8 [128,2,1]
    (an h-chunk pair of v), rhs = t8 [128,2,512] -> PSUM [1,512] per
    512-l block, 4 accumulating matmuls per block.  DVE drains PSUM to a
    [4, 4096] score tile; per-batch score rows stream back via SWDGE
    (gpsimd) so the HW rings stay dedicated to the input stream.
"""

import json

import ml_dtypes
import numpy as np

import concourse.bass as bass
import concourse.tile as tile
from concourse import bass2jax, bass_utils, mybir
from concourse.bass_utils import run_bass_kernel_spmd

F32 = mybir.dt.float32
F8 = mybir.dt.float8e4
E4 = ml_dtypes.float8_e4m3

B, L, H = 32, 4096, 1024
NCORES = 8
BPC = B // NCORES          # batches per core (4)
NCELL = 8                  # 512-l cells per batch (one PSUM block each)
NQ = 4                     # h-chunk pairs (DoubleRow: 2x128 contraction)
LB = 512                   # l per cell == PSUM bank capacity in fp32
CELLF = NQ * 2 * LB        # free elems per cell per partition (4096)
MARGIN = 24.0              # fp8 score error is sigma~1.2, max~6

# Transfer plan: (batch, cell_lo, cell_hi, queue).  2 MiB transfers for
# the bulk (fewer inter-transfer ring gaps), tapering to 512 KiB over
# the last 2 MiB so the final matmul+drain chain starts as early as
# possible.  The scalar (ACT) HWDGE ring observably starts ~3 us before
# the sync (SP) ring, so scalar leads each pair and carries the final
# transfer.  Emission order == consumption order.
TRANSFERS = [
    (0, 0, 4, "sync"), (0, 4, 8, "scalar"),
    (1, 0, 4, "sync"), (1, 4, 8, "scalar"),
    (2, 0, 4, "sync"), (2, 4, 8, "scalar"),
    (3, 0, 2, "sync"), (3, 2, 4, "scalar"),
    (3, 4, 5, "sync"), (3, 5, 6, "scalar"),
    (3, 6, 7, "sync"), (3, 7, 8, "scalar"),
]


def _split_multiwaits(bir_json):
    """The walrus build here lowers at most ONE sem-wait per instruction;
    hoist extra waits into standalone EventSemaphore instructions inserted
    just before the owner (same engine => same in-order stream)."""
    bir = json.loads(bir_json)
    for fn in bir["functions"]:
        for blk in fn["blocks"]:
            new_insts = []
            for ins in blk.get("instructions", []):
                si = ins.get("sync_info")
                ow = (si or {}).get("on_wait") or []
                if len(ow) > 1:
                    for k, w in enumerate(ow[:-1]):
                        new_insts.append(
                            {
                                "debug": ins.get("debug", 0),
                                "engine": ins["engine"],
                                "ins": [],
                                "name": f"{ins['name']}_hw{k}",
                                "opcode": "EventSemaphore",
                                "outs": [],
                                "sync_info": {"on_update": [], "on_wait": [w]},
                            }
                        )
                    si["on_wait"] = [ow[-1]]
                new_insts.append(ins)
            blk["instructions"] = new_insts
    return json.dumps(bir).encode()


_ORIG_COMPILE_BIR = bass_utils.compile_bir_kernel


def _compile_bir_split(bir_json, tmpdir, neff_name="file.neff"):
    return _ORIG_COMPILE_BIR(_split_multiwaits(bir_json), tmpdir, neff_name=neff_name)


def _patch_compile():
    bass_utils.compile_bir_kernel = _compile_bir_split
    bass2jax.compile_bir_kernel = _compile_bir_split


def _patch_tile_drain():
    """walrus in this env only lowers 1 sem-wait per TPB_CTRL Drain; split
    the TileContext exit-drain waits into individual wait_ge instructions."""
    if getattr(tile.TileContext, "_drain_patched", False):
        return

    def _drain_and_barrier(self, tick_clock, wait_clock):
        nc = self.nc
        drain_inst = nc.sync.drain()
        wait_clock.add_sem_waits(
            drain_inst.ins, tile.ScopedClock({None: tick_clock.global_clock})
        )
        si = drain_inst.ins.sync_info
        waits = list(si.on_wait or [])
        if len(waits) > 1:
            si.on_wait = []
            handles = {}
            for h in self.sems.allocated().values():
                handles[getattr(h, "name", None) or str(h)] = h
            for ww in waits:
                nc.sync.wait_ge(handles[ww.ant_name], ww.wait_value)
        nc.all_engine_barrier()
        popped = nc._tile_sem_poison_stack.pop()
        assert popped is self._sem_poison
        nc.clear_and_free_semaphores(list(self.sems.allocated().values()))
        nc.all_engine_barrier()

    tile.TileContext._drain_and_barrier = _drain_and_barrier
    tile.TileContext._drain_patched = True


def build_kernel(tc, t8d, v8d, outd):
    nc = tc.nc

    import contextlib

    ctx = contextlib.ExitStack()
    consts = ctx.enter_context(tc.tile_pool(name="consts", bufs=1))
    chp = ctx.enter_context(tc.tile_pool(name="chunks", bufs=BPC))
    sp = ctx.enter_context(tc.tile_pool(name="scores", bufs=1))
    psp = ctx.enter_context(tc.tile_pool(name="ps", bufs=6, space="PSUM"))

    # v8[p, two, b*NQ+q] = fp8(v[b, (q*2+two)*128 + p]).  The DoubleRow
    # LDWEIGHTS ISA requires the k-pair dim's step to be a multiple of 16
    # (bytes), hence pair-partner columns 16 apart rather than adjacent.
    v8 = consts.tile([128, 2, BPC * NQ], F8)
    nc.gpsimd.dma_start(out=v8, in_=v8d.rearrange("p (t i) -> p t i", t=2))
    # One score row per batch, each on partition 0 (engine APs must start
    # at a 32-aligned partition, so a [BPC, L] tile with per-batch rows
    # fails BIR verification).
    S = [sp.tile([1, L], F32, name=f"S{b}") for b in range(BPC)]

    # One resident tile per batch (4 MiB each, 16 MiB total); the DMA
    # transfers write disjoint cell ranges and the Tile framework
    # range-tracks, so matmuls wait only on the slice they read.
    tg = [
        chp.tile([128, NCELL, NQ, 2, LB], F8, tag="tg", name=f"t{b}")
        for b in range(BPC)
    ]
    for b, lo, hi, qname in TRANSFERS:
        eng = nc.sync if qname == "sync" else nc.scalar
        eng.dma_start(
            out=tg[b][:, lo:hi],
            in_=t8d[b][:, lo * CELLF : hi * CELLF].rearrange(
                "p (c q t l) -> p c q t l", q=NQ, t=2, l=LB
            ),
        )

    for b in range(BPC):
        for cell in range(NCELL):
            ps = psp.tile([1, LB], F32, tag="ps", name=f"ps{b}_{cell}")
            for q in range(NQ):
                idx = b * NQ + q
                nc.tensor.matmul(
                    ps,
                    lhsT=v8[:, :, idx : idx + 1],
                    rhs=tg[b][:, cell, q],
                    start=(q == 0),
                    stop=(q == NQ - 1),
                    perf_mode=mybir.MatmulPerfMode.DoubleRow,
                )
            col = cell * LB
            # All PSUM drains on DVE: the ACT engine issues the scalar
            # ring's DMAs, and a ring-capacity-blocked DMA issue would
            # head-of-line-block ACT copies (v4 regression: stalled
            # drains -> full PSUM pool -> stalled PE).
            nc.vector.tensor_copy(S[b][:, col : col + LB], ps)
        # Batches 0-2 stream their score rows out on the idle SWDGE path
        # (done long before the tail); batch 3 goes on the scalar HWDGE
        # ring (empty right after its final chunk) in two pieces so the
        # end-gating store is only the last cell's 2 KiB.
        if b < BPC - 1:
            nc.gpsimd.dma_start(out=outd[b], in_=S[b][:, :])
    cut = (NCELL - 1) * LB
    nc.scalar.dma_start(out=outd[BPC - 1][0:cut], in_=S[BPC - 1][:, 0:cut])
    nc.scalar.dma_start(out=outd[BPC - 1][cut:L], in_=S[BPC - 1][:, cut:L])
    ctx.close()


def build_bass():
    _patch_tile_drain()
    _patch_compile()
    nc = bass.Bass("TRN2", target_bir_lowering=False, debug=False, num_devices=NCORES)
    t8d = nc.dram_tensor(
        "t8", [BPC, 128, NCELL * CELLF], F8, kind="ExternalInput"
    ).ap()
    v8d = nc.dram_tensor("v8", [128, BPC * NQ * 2], F8, kind="ExternalInput").ap()
    outd = nc.dram_tensor("out", [BPC, L], F32, kind="ExternalOutput").ap()
    with tile.TileContext(nc) as tc:
        build_kernel(tc, t8d, v8d, outd)
    return nc


def make_in_maps(hidden, targets, W):
    h64 = hidden[0].astype(np.float64)                    # [B, H]
    v8 = (h64 @ W.astype(np.float64)).astype(np.float32).astype(E4)  # [B, H]

    in_maps = []
    for c in range(NCORES):
        tl = targets[c * BPC : (c + 1) * BPC]             # [4, 4096, 1024] f32
        t8 = tl.astype(E4)
        # l = cell*512 + l' ; h = (q*2+t)*128 + p
        t8r = t8.reshape(BPC, NCELL, LB, NQ, 2, 128)      # [b,cell,l',q,t,p]
        t8r = np.ascontiguousarray(t8r.transpose(0, 5, 1, 3, 4, 2))
        t8c = t8r.reshape(BPC, 128, NCELL * CELLF)
        vloc = v8[c * BPC : (c + 1) * BPC]                # [4, 1024]
        # v8c[p, t*16 + b*NQ + q] = vloc[b, (q*2+t)*128 + p]
        v8c = np.ascontiguousarray(
            vloc.reshape(BPC, NQ, 2, 128).transpose(3, 2, 0, 1)
        ).reshape(128, 2 * BPC * NQ)
        in_maps.append({"t8": t8c, "v8": v8c})
    return in_maps


_CACHED_NC = None


def kernel(hidden, targets, W, b, _trace=False):
    global _CACHED_NC
    if _CACHED_NC is None:
        _CACHED_NC = build_bass()
    nc = _CACHED_NC
    in_maps = make_in_maps(hidden, targets, W)
    res = run_bass_kernel_spmd(nc, in_maps, list(range(NCORES)), trace=_trace)
    s8 = np.concatenate([res.results[c]["out"] for c in range(NCORES)], axis=0)
    kernel.last_results = res

    # Host: exact rescore of candidate rows (those within MARGIN of the
    # row max -- typically ~10 of 4096) + float64 softmax.
    h64 = hidden[0].astype(np.float64)
    v64 = h64 @ W.astype(np.float64)                      # [B, H]
    out = np.empty((B, 1, L), np.float32)
    sc = s8.astype(np.float64)
    for bb in range(B):
        row = sc[bb]
        cand = np.flatnonzero(row >= row.max() - MARGIN)
        row[cand] = targets[bb, cand].astype(np.float64) @ v64[bb]
        e = np.exp(row - row.max())
        out[bb, 0] = (e / e.sum()).astype(np.float32)
    return out
